# revision 24
# baseline (speedup 1.0000x reference)
"""2-layer GCN (GCNConv -> ReLU -> GCNConv -> ReLU) on 8 Trainium2 NeuronCores.

Math: out = relu(A_hat @ relu(A_hat @ X @ W1 + b1) @ W2 + b2),
A_hat = D^-1/2 (A + I) D^-1/2.  Associativity lets us aggregate in the
input feature space of each layer: A_hat @ (X W) == (A_hat @ X) W, so every
aggregated message is 128 features wide.

Distribution: destination nodes (and their in-edges) are sharded across the
8 cores.  Per dst block of 128 rows, the segment-sum is a PE matmul chain:
aggT[f,d] += M_chunk[e,f] (contract e) S_chunk[e,d], where S is a 0/1
selection matrix (one column per dst slot) and M is the chunk's 128
source-row messages.  Self-loop terms are added with one identity matmul
from the core's own table block.  dinv scaling is folded into the tables
(dinv[src], on write) and into the ACT-engine relu readout (dinv[dst]).

v2 changes vs the first working kernel:
 - S is generated ON DEVICE (DVE iota==drel one-hot, batched 16 chunks per
   instruction) instead of streaming 27 MB x 2 of precomputed columns.
 - Chunks are bucketed by source PIECE (which half of each core's rows the
   source lives in) instead of the int16 lo/hi hack; the AllGather is split
   into two piece collectives so piece-0 L2 gathers overlap L1's second
   half and the piece-1 collective.
 - L2 dma_gathers are spread across 4 SWDGE queues (piece0 -> q0/q1,
   piece1 -> q2/q3) so in-order queues don't head-of-line block.
 - PSUM readouts (aggT copy, relu+scale) moved to the ACT engine, keeping
   DVE free for S generation and the tensor engine continuously fed.
"""

import os
import sys

for _p in ("/opt/trn_rl_repo", "/root/.axon_site/_ro/trn_rl_repo"):
    if os.path.isdir(_p) and _p not in sys.path:
        sys.path.append(_p)

import numpy as np
import ml_dtypes

import concourse.bacc as bacc
import concourse.bass as bass
import concourse.tile as tile
from concourse import mybir
from concourse.bass_utils import run_bass_kernel_spmd

NC = 8          # cores
BLK = 128       # dst rows per aggregation block
CHUNK = 128     # edges per matmul chunk (PE contraction dim)
NPIECE = 2      # source-piece buckets == piece-wise AllGather count
GB = 16         # chunks per msg-stream DMA / S-generation batch
G = 16          # chunks per dma_gather call (layer 2)
GAT0_BUFS = 16  # piece-0 gather tiles in flight
GAT1_BUFS = 8
PRE_G0 = 16     # piece-0 gathers pre-emitted before the piece-1 collective
DEFER = 49      # L2 blocks whose piece-1 chunks run in a second pass


def _pack_idx(flat: np.ndarray) -> np.ndarray:
    """dma_gather index layout: [128, n/16] int16, idx i at [i%16, i//16],
    replicated across the 8 gpsimd cores (partition groups of 16)."""
    n = flat.shape[0]
    assert n % 16 == 0
    return np.ascontiguousarray(np.tile(flat.reshape(n // 16, 16).T, (8, 1)))


class _Plan:
    """Host-side schedule + per-core streams (shared by both layers)."""

    def __init__(self, n_nodes: int, edge_index: np.ndarray):
        assert n_nodes % (NC * NPIECE) == 0
        self.N = n_nodes
        self.ROWS = n_nodes // NC
        self.PSTEP = self.ROWS // NPIECE
        self.PIECE_ROWS = NC * self.PSTEP
        self.NBLK = (self.ROWS + BLK - 1) // BLK
        self.last_rows = self.ROWS - (self.NBLK - 1) * BLK

        src = np.asarray(edge_index[0], dtype=np.int64)
        dst = np.asarray(edge_index[1], dtype=np.int64)

        # self-loops are NOT materialized as edge slots: each block adds its
        # own table rows via an identity matmul.  deg still counts them.
        deg = (np.bincount(dst, minlength=n_nodes) + 1).astype(np.float64)
        self.dinv = (1.0 / np.sqrt(deg)).astype(np.float32)

        core = dst // self.ROWS
        rem = dst % self.ROWS
        blk = rem // BLK
        drel = rem % BLK
        # source piece + row within the piece-major AllGather table
        spc = src // self.ROWS
        srem = src % self.ROWS
        piece = srem // self.PSTEP
        srow = spc * self.PSTEP + (srem % self.PSTEP)  # [0, PIECE_ROWS)
        assert self.PIECE_ROWS <= 32768  # int16 gather indices

        key = (core * self.NBLK + blk) * NPIECE + piece
        counts = np.bincount(
            key, minlength=NC * self.NBLK * NPIECE
        ).reshape(NC, self.NBLK, NPIECE)
        K = -(-counts // CHUNK)
        K = K.max(axis=0)  # [NBLK, NPIECE] chunk counts, shared by all cores
        self.Kp = [K[:, p].astype(np.int64) for p in range(NPIECE)]
        self.C_p = [int(k.sum()) for k in self.Kp]
        self.C = sum(self.C_p)

        base = lambda k: np.concatenate([[0], np.cumsum(k)])
        self.base_p = [base(k) for k in self.Kp]
        self.base_g = base(sum(self.Kp))

        # per-core streams (piece-stream indexed, so layer-2 can defer
        # piece-1 chunks without breaking tile-pool consumption order)
        self.drel_p = []   # NPIECE x [128, C_p] int16 one-hot cols (-1 pad)
        self.slotsrc = []  # [C*128] int64 source node per slot (pad: 0)
        self.idxp = []     # NPIECE x [128, C_p*8] int16 gather indices
        self.dinv_col = []   # [128, NBLK] f32
        self.dinv2_col = []  # [128, NBLK] f32
        for c in range(NC):
            m = core == c
            sg, sr, bb, dd, ss = src[m], srow[m], blk[m], drel[m], piece[m]
            k2 = bb * NPIECE + ss
            order = np.argsort(k2, kind="stable")
            sg, sr, bb, dd, ss, k2 = (
                sg[order], sr[order], bb[order], dd[order], ss[order],
                k2[order],
            )
            change = np.r_[True, k2[1:] != k2[:-1]]
            startidx = np.flatnonzero(change)
            sizes = np.diff(np.r_[startidx, len(k2)])
            pos = np.arange(len(k2)) - np.repeat(startidx, sizes)
            ch = pos // CHUNK
            lane = pos % CHUNK
            q = self.base_g[bb] + np.where(
                ss == 0, ch, self.Kp[0][bb] + ch
            )
            pstream = np.where(
                ss == 0, self.base_p[0][bb], self.base_p[1][bb]
            ) + ch

            drel_p = []
            for p in range(NPIECE):
                dr = np.full((128, max(self.C_p[p], 1)), -1, dtype=np.int16)
                pm = ss == p
                dr[lane[pm], pstream[pm]] = dd[pm]
                drel_p.append(np.ascontiguousarray(dr))
            self.drel_p.append(drel_p)

            slotsrc = np.zeros(self.C * CHUNK, np.int64)
            slotsrc[q * CHUNK + lane] = sg
            self.slotsrc.append(slotsrc)

            idxp = []
            for p in range(NPIECE):
                ix = np.zeros(max(self.C_p[p], 1) * CHUNK, np.int16)
                pm = ss == p
                ix[pstream[pm] * CHUNK + lane[pm]] = sr[pm].astype(np.int16)
                idxp.append(_pack_idx(ix))
            self.idxp.append(idxp)

            dv = np.ones(self.NBLK * BLK, np.float32)
            dv[: self.ROWS] = self.dinv[c * self.ROWS : (c + 1) * self.ROWS]
            dv = dv.reshape(self.NBLK, BLK).T.copy()  # [128, NBLK]
            self.dinv_col.append(dv)
            self.dinv2_col.append(dv * dv)

    def msg1(self, xg_full: np.ndarray, c: int) -> np.ndarray:
        """Layer-1 message stream for core c, already in SBUF layout
        [128 slot-lane partitions, C*128 (chunk-major, feature-minor)]."""
        m = xg_full[self.slotsrc[c]]  # [C*128, F]
        F = m.shape[1]
        return np.ascontiguousarray(
            m.reshape(self.C, CHUNK, F).transpose(1, 0, 2).reshape(
                128, self.C * F
            )
        )

    def signature(self, f_in, f_out, has_b1, has_b2):
        return (
            self.N, f_in, f_out, has_b1, has_b2,
            tuple(self.Kp[0]), tuple(self.Kp[1]),
        )


def _build(plan: _Plan, f_in: int, f_out: int, has_b1: bool, has_b2: bool):
    """Build + compile the SPMD Bass program (one NEFF, runs on all 8 cores)."""
    ROWS, NBLK = plan.ROWS, plan.NBLK
    C = plan.C
    PSTEP, PIECE_ROWS = plan.PSTEP, plan.PIECE_ROWS
    bf16, f32, i16 = mybir.dt.bfloat16, mybir.dt.float32, mybir.dt.int16

    nc = bacc.Bacc("TRN2", target_bir_lowering=False, debug=False,
                   enable_asserts=True, num_devices=NC, num_swdge_queues=4)

    xgl = nc.dram_tensor("xgl", [ROWS, f_in], bf16, kind="ExternalInput")
    msg1 = nc.dram_tensor("msg1", [128, C * f_in], bf16, kind="ExternalInput")
    ident_in = nc.dram_tensor("ident", [128, 128], bf16, kind="ExternalInput")
    w1 = nc.dram_tensor("w1", [f_in, f_in], f32, kind="ExternalInput")
    w2 = nc.dram_tensor("w2", [f_in, f_out], f32, kind="ExternalInput")
    if has_b1:
        b1bc = nc.dram_tensor("b1bc", [128, f_in], f32, kind="ExternalInput")
    if has_b2:
        b2bc = nc.dram_tensor("b2bc", [128, f_out], f32, kind="ExternalInput")
    drel_d = [
        nc.dram_tensor(f"drel{p}", [128, max(plan.C_p[p], 1)], i16,
                       kind="ExternalInput")
        for p in range(NPIECE)
    ]
    idxp_d = [
        nc.dram_tensor(f"idxp{p}", [128, max(plan.C_p[p], 1) * 8], i16,
                       kind="ExternalInput")
        for p in range(NPIECE)
    ]
    dinv_c = nc.dram_tensor("dinv_c", [128, NBLK], f32, kind="ExternalInput")
    dinv2_c = nc.dram_tensor("dinv2_c", [128, NBLK], f32, kind="ExternalInput")
    out_ext = nc.dram_tensor("out", [ROWS, f_out], f32, kind="ExternalOutput")

    with tile.TileContext(nc) as tc:
        with (
            tc.tile_pool(name="meta", bufs=1) as pm,
            tc.tile_pool(name="work", bufs=2) as pw,
            tc.tile_pool(name="psum", space="PSUM", bufs=2) as pp,
            tc.tile_pool(name="dram", space="DRAM", bufs=1) as pd,
        ):
            # ---- persistent metadata in SBUF ----
            ident_t = pm.tile([128, 128], bf16)
            nc.sync.dma_start(ident_t[:], ident_in[:])
            idx_t = []
            for p in range(NPIECE):
                t = pm.tile([128, max(plan.C_p[p], 1) * 8], i16,
                            name=f"idxt{p}", tag=f"idxt{p}")
                nc.sync.dma_start(t[:], idxp_d[p][:])
                idx_t.append(t)
            drel_t = []
            for p in range(NPIECE):
                t = pm.tile([128, max(plan.C_p[p], 1)], i16,
                            name=f"drelt{p}", tag=f"drelt{p}")
                nc.sync.dma_start(t[:], drel_d[p][:])
                drel_t.append(t)
            dinv_t = pm.tile([128, NBLK], f32)
            nc.sync.dma_start(dinv_t[:], dinv_c[:])
            dinv2_t = pm.tile([128, NBLK], f32)
            nc.sync.dma_start(dinv2_t[:], dinv2_c[:])

            iota_t = pm.tile([128, 128], i16, name="iota")
            nc.gpsimd.iota(iota_t[:], pattern=[[1, 128]], base=0,
                           channel_multiplier=0)

            w1f = pm.tile([f_in, f_in], f32)
            nc.sync.dma_start(w1f[:], w1[:])
            w1_t = pm.tile([f_in, f_in], bf16)
            nc.scalar.activation(w1_t[:], w1f[:],
                                 mybir.ActivationFunctionType.Copy)
            w2f = pm.tile([f_in, f_out], f32)
            nc.sync.dma_start(w2f[:], w2[:])
            w2_t = pm.tile([f_in, f_out], bf16)
            nc.scalar.activation(w2_t[:], w2f[:],
                                 mybir.ActivationFunctionType.Copy)
            if has_b1:
                b1_t = pm.tile([128, f_in], f32)
                nc.sync.dma_start(b1_t[:], b1bc[:])
            if has_b2:
                b2_t = pm.tile([128, f_out], f32)
                nc.sync.dma_start(b2_t[:], b2bc[:])

            # The DMAGatherAnt instruction struct only has room for ONE sync
            # wait command (walrus setupSyncWait limit).  Absorb the idx-load
            # dependencies into a throwaway gpsimd DMA so every dma_gather
            # needs at most one wait (collective done / msg-buf WAR).
            scratch = pm.tile([1, 16], i16, name="scratch")
            for p in range(NPIECE):
                nc.gpsimd.dma_start(scratch[0:1, 0:16], idx_t[p][0:1, 0:16])

            ag2_in = pd.tile([ROWS, f_in], bf16)
            h1_tab = [
                pd.tile([PIECE_ROWS, f_in], bf16, addr_space="Shared",
                        name=f"h1_p{p}")
                for p in range(NPIECE)
            ]

            # ---- S generation: one-hot columns from drel via iota==drel,
            # batched GB chunks per DVE op, keyed by (layer, piece, group)
            # so consumption order stays monotonic per piece stream ----
            s_tiles = {}

            def get_s(layer, p, pos):
                j, col = divmod(pos, GB)
                key = (layer, p, j)
                if key not in s_tiles:
                    n = min(GB, plan.C_p[p] - j * GB)
                    st = pw.tile([128, GB * 128], bf16, tag=f"sel{p}",
                                 bufs=6, name=f"sel{layer}_{p}_{j}")
                    out_ap = st[:, : n * 128].rearrange(
                        "p (g e) -> p g e", e=128)
                    i0 = iota_t[:, :].rearrange("p (g e) -> p g e", g=1)
                    i1 = drel_t[p][:, j * GB : j * GB + n].rearrange(
                        "p (g e) -> p g e", e=1)
                    i0b, i1b = bass.broadcast_tensor_aps(i0, i1)
                    nc.vector.tensor_tensor(
                        out=out_ap, in0=i0b, in1=i1b,
                        op=mybir.AluOpType.is_equal)
                    s_tiles[key] = st
                return s_tiles[key], col

            # ---- layer-1 message stream (host-materialized, global chunk
            # order: per block, piece-0 chunks then piece-1 chunks) ----
            msg_tiles = {}

            def get_msg1(b, p, i):
                q = int(plan.base_g[b]) + (
                    i if p == 0 else int(plan.Kp[0][b]) + i
                )
                j, col = divmod(q, GB)
                if j not in msg_tiles:
                    n = min(GB, C - j * GB)
                    t = pw.tile([128, GB * f_in], bf16, tag="msg", bufs=5,
                                name=f"msg_{j}")
                    nc.sync.dma_start(
                        t[:, : n * f_in],
                        msg1[:, j * GB * f_in : (j * GB + n) * f_in])
                    msg_tiles[j] = t
                return msg_tiles[j], col

            # ---- layer-2 gathered message tiles ----
            gat_tiles = {}
            gq_rr = [0, 0]

            def emit_gather(s, j):
                cs = plan.C_p[s]
                n = min(G, cs - j * G)
                mt = pw.tile([128, G * CHUNK], bf16, tag=f"gat{s}",
                             bufs=(GAT0_BUFS if s == 0 else GAT1_BUFS),
                             name=f"gat{s}_{j}")
                view = h1_tab[s][:, :]
                qn = 2 * s + (gq_rr[s] % 2)
                gq_rr[s] += 1
                nc.gpsimd.dma_gather(
                    mt[:, : n * CHUNK].rearrange("p (g e) -> p g e", e=f_in),
                    view,
                    idx_t[s][:, j * G * 8 : (j * G + n) * 8],
                    n * CHUNK, n * CHUNK, f_in,
                    single_packet=False, queue_num=qn,
                )
                gat_tiles[(s, j)] = mt

            def get_msg2(b, p, i):
                pos = int(plan.base_p[p][b]) + i
                j, col = divmod(pos, G)
                if (p, j) not in gat_tiles:
                    emit_gather(p, j)
                return gat_tiles[(p, j)], col

            def coll(p):
                nc.gpsimd.collective_compute(
                    "AllGather", mybir.AluOpType.bypass,
                    replica_groups=[list(range(NC))],
                    ins=[ag2_in[p * PSTEP : (p + 1) * PSTEP, :]],
                    outs=[h1_tab[p][:, :]],
                )

            # the L1 block whose output DMA completes piece 0 of ag2_in
            coll0_block = (PSTEP + BLK - 1) // BLK - 1

            def open_block(layer, b, local_tab, pieces, tagsuf=""):
                """Start an aggregation psum: self-loop + message chunks
                for the given pieces.  Returns (psum, had_stop)."""
                rows = plan.last_rows if b == NBLK - 1 else BLK
                nch = sum(int(plan.Kp[p][b]) for p in pieces)
                psum = pp.tile([128, 128], f32, tag="agg", bufs=6,
                               name=f"agg_l{layer}_{b}{tagsuf}")
                # self-loop contribution: psum[f,d] += local[dd,f]*I[dd,d]
                loc = pw.tile([128, 128], bf16, tag="loc", bufs=4,
                              name=f"loc_l{layer}_{b}{tagsuf}")
                nc.scalar.dma_start(
                    loc[:rows, :],
                    local_tab[b * BLK : b * BLK + rows, :],
                )
                # rows < BLK leaves stale data in loc[rows:]; identity
                # routing sends row dd only to psum column dd, and
                # columns >= rows are never read back, so it's harmless.
                nc.tensor.matmul(psum[:], lhsT=loc[:], rhs=ident_t[:],
                                 start=True, stop=(nch == 0))
                add_chunks(layer, b, psum, pieces, nch)
                return psum

            def add_chunks(layer, b, psum, pieces, nch, done=0):
                get_msg = get_msg1 if layer == 0 else get_msg2
                k = done
                for p in pieces:
                    for i in range(int(plan.Kp[p][b])):
                        mt, mcol = get_msg(b, p, i)
                        st, scol = get_s(layer, p,
                                         int(plan.base_p[p][b]) + i)
                        k += 1
                        nc.tensor.matmul(
                            psum[:],
                            lhsT=mt[:, mcol * CHUNK : (mcol + 1) * CHUNK],
                            rhs=st[:, scol * 128 : (scol + 1) * 128],
                            start=False, stop=(k == nch),
                        )

            def read_block(layer, b, psum, w_t, fo, bias_t, scale_t,
                           out_tab):
                rows = plan.last_rows if b == NBLK - 1 else BLK
                aggT = pw.tile([128, 128], bf16, tag="aggT", bufs=4,
                               name=f"aggT_l{layer}_{b}")
                nc.scalar.activation(aggT[:], psum[:],
                                     mybir.ActivationFunctionType.Copy)
                ph = pp.tile([128, fo], f32, tag="hout", bufs=2,
                             name=f"ph_l{layer}_{b}")
                nc.tensor.matmul(ph[:], lhsT=aggT[:], rhs=w_t[:],
                                 start=True, stop=True)
                hsb = pw.tile([128, fo], bf16 if layer == 0 else f32,
                              tag=f"hsb{layer}", bufs=4,
                              name=f"hsb_l{layer}_{b}")
                if bias_t is None:
                    # relu(dinv*x)*k == relu(dinv*k*x): one ACT op
                    nc.scalar.activation(
                        hsb[:], ph[:],
                        mybir.ActivationFunctionType.Relu,
                        scale=scale_t[:, b : b + 1],
                    )
                else:
                    tmp = pw.tile([128, fo], f32, tag=f"tmp{layer}",
                                  bufs=2, name=f"tmp_l{layer}_{b}")
                    nc.vector.scalar_tensor_tensor(
                        out=tmp[:], in0=ph[:],
                        scalar=dinv_t[:, b : b + 1], in1=bias_t[:],
                        op0=mybir.AluOpType.mult,
                        op1=mybir.AluOpType.add,
                    )
                    if layer == 0:
                        nc.vector.tensor_scalar(
                            out=hsb[:], in0=tmp[:], scalar1=0.0,
                            scalar2=dinv_t[:, b : b + 1],
                            op0=mybir.AluOpType.max,
                            op1=mybir.AluOpType.mult,
                        )
                    else:
                        nc.vector.tensor_scalar(
                            out=hsb[:], in0=tmp[:], scalar1=0.0,
                            scalar2=None,
                            op0=mybir.AluOpType.max,
                            op1=mybir.AluOpType.bypass,
                        )
                dst_rows = slice(b * BLK, b * BLK + rows)
                nc.scalar.dma_start(out_tab[dst_rows, :], hsb[:rows, :])

            # ---- layer 1: table rows pre-scaled by dinv[src] on host;
            # output row d is relu(dinv_d*(agg@W1)+b1)*dinv_d (L2 table) ----
            b1t = b1_t if has_b1 else None
            for b in range(NBLK):
                psum = open_block(0, b, xgl, range(NPIECE))
                read_block(0, b, psum, w1_t, f_in, b1t, dinv2_t, ag2_in)
                if b == coll0_block:
                    # piece-0 rows of ag2_in are written: publish them and
                    # pre-issue piece-0 gathers (they run during L1's tail).
                    coll(0)
                    for j in range(
                        min(PRE_G0, (plan.C_p[0] + G - 1) // G)
                    ):
                        emit_gather(0, j)
            coll(1)

            # ---- layer 2: out row d = relu(dinv_d * (agg @ W2) + b2).
            # The first DEFER blocks run piece-0 chunks while the piece-1
            # collective is still in flight; their partials park in SBUF
            # and resume via an identity matmul once piece 1 lands. ----
            b2t = b2_t if has_b2 else None
            D = min(DEFER, NBLK)
            aggP = []
            for b in range(D):
                nch = int(plan.Kp[0][b]) + int(plan.Kp[1][b])
                psum = open_block(1, b, ag2_in, (0,), tagsuf="a")
                t = pw.tile([128, 128], bf16, tag="aggP", bufs=D,
                            name=f"aggP_{b}")
                nc.scalar.activation(t[:], psum[:],
                                     mybir.ActivationFunctionType.Copy)
                aggP.append(t)
            for b in range(D):
                nch = int(plan.Kp[1][b])
                psum = pp.tile([128, 128], f32, tag="agg", bufs=6,
                               name=f"agg_l1_{b}b")
                # identity stationary, aggP moving: psum[f,d] = aggP[f,d]
                # (lhsT=aggP would transpose it)
                nc.tensor.matmul(psum[:], lhsT=ident_t[:], rhs=aggP[b][:],
                                 start=True, stop=(nch == 0))
                add_chunks(1, b, psum, (1,), nch)
                read_block(1, b, psum, w2_t, f_out, b2t, dinv_t, out_ext)
            for b in range(D, NBLK):
                psum = open_block(1, b, ag2_in, range(NPIECE))
                read_block(1, b, psum, w2_t, f_out, b2t, dinv_t, out_ext)

    nc.compile()
    return nc


_cache: dict = {}


def _get_program(plan, f_in, f_out, has_b1, has_b2):
    key = plan.signature(f_in, f_out, has_b1, has_b2)
    if key not in _cache:
        _cache[key] = _build(plan, f_in, f_out, has_b1, has_b2)
    return _cache[key]


def _run(x, edge_index, W1, b1, W2, b2, trace=False, trace_cores=None):
    x = np.asarray(x, dtype=np.float32)
    W1 = np.asarray(W1, dtype=np.float32)
    W2 = np.asarray(W2, dtype=np.float32)
    b1 = np.asarray(b1, dtype=np.float32)
    b2 = np.asarray(b2, dtype=np.float32)
    N, f_in = x.shape
    f_out = W2.shape[1]
    has_b1 = bool(np.any(b1))
    has_b2 = bool(np.any(b2))

    plan = _Plan(N, np.asarray(edge_index))
    nc = _get_program(plan, f_in, f_out, has_b1, has_b2)

    in_maps = _make_in_maps(plan, x, W1, b1, W2, b2)

    kw = {}
    if trace:
        kw = dict(trace=True)
        if trace_cores is not None:
            kw["trace_cores"] = trace_cores
    res = run_bass_kernel_spmd(nc, in_maps, core_ids=list(range(NC)), **kw)
    out = np.concatenate([res.results[c]["out"] for c in range(NC)], axis=0)
    return out, res


def _make_in_maps(plan, x, W1, b1, W2, b2):
    has_b1 = bool(np.any(b1))
    has_b2 = bool(np.any(b2))
    xg_full = (x * plan.dinv[:, None]).astype(ml_dtypes.bfloat16)
    ident = np.eye(128, dtype=ml_dtypes.bfloat16)
    in_maps = []
    for c in range(NC):
        m = dict(
            xgl=np.ascontiguousarray(
                xg_full[c * plan.ROWS : (c + 1) * plan.ROWS]
            ),
            msg1=plan.msg1(xg_full, c),
            ident=ident,
            w1=W1, w2=W2,
            dinv_c=plan.dinv_col[c],
            dinv2_c=plan.dinv2_col[c],
        )
        for p in range(NPIECE):
            m[f"idxp{p}"] = plan.idxp[c][p]
            m[f"drel{p}"] = plan.drel_p[c][p]
        if has_b1:
            m["b1bc"] = np.ascontiguousarray(np.tile(b1, (128, 1)))
        if has_b2:
            m["b2bc"] = np.ascontiguousarray(np.tile(b2, (128, 1)))
        in_maps.append(m)
    return in_maps


def kernel(x, edge_index, W1, b1, W2, b2):
    out, _ = _run(x, edge_index, W1, b1, W2, b2)
    return out
